# revision 29
# baseline (speedup 1.0000x reference)
"""Trainium2 Bass kernel for a dense transformer decoder layer.

Model: B=2, S=2048, H=2048, NH=16, HD=128, FF=8192, fp32 I/O.

Sharding (8 NeuronCores): DP-2 over batch x seq-DP-2 (even/odd token
interleave) across HBM-pairs x TP-2 over heads / FF inside each HBM pair.

  core c: pair p=c//2, head-half hh=c%2; batch b=p//2, parity par=p%2.
  The pair handles the 1024 tokens of batch b at positions par::2.
  Each core owns 8 heads (column half of wq/wk/wv, row half of wo) and
  half of FF.  K/V for all 2048 batch tokens are computed locally
  (replicated inside the batch); the only cross-core traffic is the
  o_proj / down_proj partial-sum exchange between the two cores of an
  HBM pair through pair-shared DRAM, with tiny 2-rank AllGather
  barriers.

v3: the whole attention side (K/V/Q projections, attn@V, o_proj) runs
in fp8e4 with DoubleRow perf mode (2 fp8 k-rows per PE pass = 2x the
bf16 matmul rate).  Weights wq/wk/wv/wo are pre-scaled x32 into fp8 on
the host; the 32x factors cancel through the softmax ratio and fold
into the exp scale / o-fold scalars (o folds x/2 via the pre-halved
xtoh input).  The MLP stays bf16 (fp8 there fails the 2e-2 gate) and
runs pass0 -> down0 -> pass1 -> down1 so aT needs only one 512-token
buffer and the chunk-0 exchange overlaps pass1.  rmsnorm squares ride
the Activation engine (Square), PSUM->SBUF copies ride Activation
(Copy), rstd uses the fused Rsqrt activation, and wq/wo/x-own tiles
prefetch during phase A.  The down-proj chunk-1 barriers interleave
into its matmul stream so the finalize tail overlaps compute.
"""

import sys

sys.path.insert(0, "/opt/trn_rl_repo")

import contextlib

import numpy as np

import concourse.bass as bass
import concourse.tile as tile
from concourse import bacc, mybir
from concourse.bass_utils import run_bass_kernel_spmd

dt = mybir.dt

B, S, H = 2, 2048, 2048
NH, HD = 16, 128
FF = 8192
EPS = 1e-6
N_CORES = 8

TOK = S // 2          # own tokens per pair (1024)
HH = H // 2           # per-core head columns (1024)
FFH = FF // 2         # per-core FF (4096)
NHT = H // 128        # 16
NFT = FFH // 128      # 32
HT = TOK // 2         # 512: exchange chunk
QC = 256              # attention query chunk
NQC = TOK // QC       # 4
NKB = S // 128        # 16 key blocks
WS = 32.0             # host-side fp8 weight scale for wq/wk/wv/wo
SCALE = 1.0 / float(np.sqrt(HD))
PAIRS = [[0, 1], [2, 3], [4, 5], [6, 7]]

Exp = mybir.ActivationFunctionType.Exp
Silu = mybir.ActivationFunctionType.Silu
Sqrt = mybir.ActivationFunctionType.Sqrt
Copy = mybir.ActivationFunctionType.Copy
Square = mybir.ActivationFunctionType.Square
MUL = mybir.AluOpType.mult
ADD = mybir.AluOpType.add
DR = mybir.MatmulPerfMode.DoubleRow
F8 = dt.float8e4


def _rt(ap):
    """[T*128, C] -> [128, T, C] (tile index as middle axis)."""
    return ap.rearrange("(t p) c -> p t c", p=128)


PHASE_MARKS = []


def build_nc():
    nc = bacc.Bacc(None, num_devices=N_CORES)

    def mark(label):
        PHASE_MARKS.append((label, nc.next_id()))

    # ---------------- I/O ----------------
    xt_e = nc.dram_tensor("xtb", [H, S], dt.bfloat16, kind="ExternalInput")
    xto_e = nc.dram_tensor("xtob", [H, TOK], dt.bfloat16, kind="ExternalInput")
    xth_e = nc.dram_tensor("xtoh", [H, TOK], dt.bfloat16, kind="ExternalInput")
    mk_e = nc.dram_tensor("mkd", [128, 4 * QC], dt.float32, kind="ExternalInput")
    wq_e = nc.dram_tensor("wq", [H, HH], F8, kind="ExternalInput")
    wk_e = nc.dram_tensor("wk", [H, HH], F8, kind="ExternalInput")
    wv_e = nc.dram_tensor("wv", [H, HH], F8, kind="ExternalInput")
    wo_e = nc.dram_tensor("wo", [HH, H], F8, kind="ExternalInput")
    wg_e = nc.dram_tensor("wg", [H, FFH], dt.bfloat16, kind="ExternalInput")
    wu_e = nc.dram_tensor("wu", [H, FFH], dt.bfloat16, kind="ExternalInput")
    wd_e = nc.dram_tensor("wd", [FFH, H], dt.bfloat16, kind="ExternalInput")
    gi_e = nc.dram_tensor("g_in", [H, 1], dt.float32, kind="ExternalInput")
    gp_e = nc.dram_tensor("g_post", [H, 1], dt.float32, kind="ExternalInput")
    slot_e = nc.dram_tensor("slot", [1, 2], dt.uint32, kind="ExternalInput")
    out_e = nc.dram_tensor("out", [H, TOK], dt.bfloat16, kind="ExternalOutput")
    # ---------------- internal DRAM ----------------
    xo_d = nc.dram_tensor("xo_d", [2, 128, NHT * TOK], dt.bfloat16, addr_space="Shared")
    xd_d = nc.dram_tensor("xd_d", [2, 128, NHT * TOK], dt.bfloat16, addr_space="Shared")
    b1i_d = nc.dram_tensor("b1i_d", [1, 1], dt.float32)
    b1o_d = nc.dram_tensor("b1o_d", [1, 2], dt.float32)
    b2i_d = nc.dram_tensor("b2i_d", [1, 1], dt.float32)
    b2o_d = nc.dram_tensor("b2o_d", [1, 2], dt.float32)
    b3i_d = nc.dram_tensor("b3i_d", [1, 1], dt.float32)
    b3o_d = nc.dram_tensor("b3o_d", [1, 2], dt.float32)
    b4i_d = nc.dram_tensor("b4i_d", [1, 1], dt.float32)
    b4o_d = nc.dram_tensor("b4o_d", [1, 2], dt.float32)

    xt_t = _rt(xt_e[:])
    xto_t = _rt(xto_e[:])
    xth_t = _rt(xth_e[:])
    mk_t = mk_e[:].rearrange("p (d q) -> p d q", d=4)
    wo_t = _rt(wo_e[:])
    wg_t = _rt(wg_e[:])
    wu_t = _rt(wu_e[:])
    wd_t = _rt(wd_e[:])
    gi_t = _rt(gi_e[:])
    gp_t = _rt(gp_e[:])
    out_t = _rt(out_e[:])

    with tile.TileContext(nc) as tc, contextlib.ExitStack() as top:
        glob = top.enter_context(tc.tile_pool(name="glob", bufs=1))
        ones_r = glob.tile([128, 1], dt.float32r)
        ones_pd = glob.tile([128, 1], F8)
        tmp1 = glob.tile([128, 1], dt.float32)
        nc.vector.memset(tmp1[:], 1.0)
        nc.vector.tensor_copy(ones_r[:], tmp1[:])
        tmp2 = glob.tile([128, 1], dt.float32)
        nc.vector.memset(tmp2[:], WS)
        nc.vector.tensor_copy(ones_pd[:], tmp2[:])
        eps1 = glob.tile([1, 1], dt.float32)
        nc.vector.memset(eps1[:], EPS)
        biasm2 = glob.tile([128, 1], dt.float32)
        nc.vector.memset(biasm2[:], -2.0)
        eps_ones = glob.tile([128, 1], F8)
        epsq = glob.tile([128, QC], F8)
        nc.vector.memset(tmp2[:], 2.0 ** -9)
        nc.vector.tensor_copy(eps_ones[:], tmp2[:])
        nc.vector.memset(epsq[:], 2.0 ** -9)
        ones_col = glob.tile([1, 128], dt.float32r)
        tmp5 = glob.tile([1, 128], dt.float32)
        nc.vector.memset(tmp5[:], 1.0)
        nc.vector.tensor_copy(ones_col[:], tmp5[:])
        flag = glob.tile([1, 1], dt.float32)
        nc.vector.memset(flag[:], 1.0)
        gi_sb = glob.tile([128, NHT], dt.float32)
        gp_sb = glob.tile([128, NHT], dt.float32)

        def barrier(writes, bi_d, bo_d):
            nc.sync.dma_start(out=bi_d[:], in_=flag[:])
            cc = nc.gpsimd.collective_compute(
                "AllGather", mybir.AluOpType.bypass, replica_groups=PAIRS,
                ins=[bi_d[:].opt()], outs=[bo_d[:].opt()])
            for d in writes:
                tile.add_dep_helper(cc.ins, d.ins, sync=True,
                                    reason="partial writes before barrier")
            return cc

        # chunk-0 x2 (residual), its rstd and its h2 stay SBUF-resident
        # across the whole kernel (outer pool: survives the kvq release)
        x2p0 = top.enter_context(tc.tile_pool(name="x2p0", bufs=1))
        x2c0 = x2p0.tile([128, NHT, HT], dt.bfloat16)
        rs0 = x2p0.tile([1, HT], dt.float32)
        h2e0 = x2p0.tile([128, NHT, HT], dt.bfloat16)

        # prefetch tiles that die with the attention phase: wq/wo (fp8) and
        # the phase-B chunk-0 x tile
        s_pre = contextlib.ExitStack()
        pre = s_pre.enter_context(tc.tile_pool(name="pre", bufs=1))
        wq_sb = pre.tile([128, NHT, HH], F8)
        wo_sb = pre.tile([128, 8, H], F8)
        xb0_sb = pre.tile([128, NHT, QC], dt.bfloat16)

        # attention activations: freed after phase C so the MLP can use
        # the space (pool releases must nest, hence the dedicated stack)
        s_kvq = contextlib.ExitStack()
        kvq = s_kvq.enter_context(tc.tile_pool(name="kvq", bufs=1))
        K_sb = kvq.tile([128, 8, S], F8)        # K^T (x32) per own head
        V_sb = kvq.tile([128, NKB, HH], F8)     # V (x32), token-part blocks
        Q_sb = kvq.tile([128, 8, TOK], F8)      # Q^T (x32) per own head

        mark('A')
        # ============ Phase A: rmsnorm(x) -> h8; K^T and V for all S tokens
        CH = 256
        NCH = S // CH
        with contextlib.ExitStack() as ph:
            wkv = ph.enter_context(tc.tile_pool(name="wkv", bufs=1))
            xin = ph.enter_context(tc.tile_pool(name="xin", bufs=2))
            hpool = ph.enter_context(tc.tile_pool(name="hpool", bufs=2))
            sm1 = ph.enter_context(tc.tile_pool(name="sm1", bufs=3))
            sqp = ph.enter_context(tc.tile_pool(name="sqp", bufs=10))
            psv = ph.enter_context(tc.tile_pool(name="psv", bufs=2, space="PSUM"))
            psk = ph.enter_context(tc.tile_pool(name="psk", bufs=2, space="PSUM"))

            wk_sb = wkv.tile([128, NHT, HH], F8)
            wv_sb = wkv.tile([128, NHT, HH], F8)

            def a_var(ci, x_sb):
                pvar = psv.tile([1, CH], dt.float32)
                for ht in range(NHT):
                    sq = sqp.tile([128, CH], dt.float32r)
                    if ht % 4 == 1:
                        nc.vector.tensor_mul(sq[:], x_sb[:, ht, :], x_sb[:, ht, :])
                    else:
                        nc.gpsimd.tensor_mul(sq[:], x_sb[:, ht, :], x_sb[:, ht, :])
                    nc.tensor.matmul(pvar[:], ones_r[:], sq[:],
                                     start=(ht == 0), stop=(ht == NHT - 1))
                std = sm1.tile([1, CH], dt.float32)
                nc.scalar.activation(std[:], pvar[:], Sqrt, scale=1.0 / H,
                                     bias=eps1[:])
                rstd = sm1.tile([1, CH], dt.float32r)
                with nc.allow_low_precision(reason="f32r==fp32 bits; PE bcast"):
                    nc.vector.reciprocal(rstd[:], std[:])
                return rstd

            def a_proj(ci, x_sb, rstd):
                sl = slice(ci * CH, (ci + 1) * CH)
                # partition-broadcast rstd on the PE (K=1 matmul): keeps the
                # Pool queue free of the variance chain so squares pipeline
                bc = psv.tile([128, CH], dt.float32, tag="bc")
                nc.tensor.matmul(bc[:], ones_col[:], rstd[:],
                                 start=True, stop=True)
                h_sb = hpool.tile([128, NHT, CH], F8)
                for ht in range(NHT):
                    nc.vector.scalar_tensor_tensor(
                        h_sb[:, ht, :], x_sb[:, ht, :], gi_sb[:, ht:ht + 1], bc[:],
                        MUL, MUL)
                # K^T tiles [kcol 128, CH] -> K_sb (fp8 DoubleRow)
                for kc in range(HH // 128):
                    pk = psk.tile([128, CH], dt.float32, tag="pk")
                    for t2 in range(NHT // 2):
                        nc.tensor.matmul(pk[:],
                                         wk_sb[:, 2 * t2:2 * t2 + 2,
                                               kc * 128:(kc + 1) * 128],
                                         h_sb[:, 2 * t2:2 * t2 + 2, :],
                                         start=(t2 == 0), stop=(t2 == NHT // 2 - 1),
                                         perf_mode=DR)
                    nc.scalar.activation(K_sb[:, kc, sl], pk[:], Copy)
                # V tiles [tok 128, 512] -> V_sb (fp8 DoubleRow)
                for tb in range(CH // 128):
                    for vc in range(HH // 512):
                        pv = psk.tile([128, 512], dt.float32, tag="pv")
                        for t2 in range(NHT // 2):
                            nc.tensor.matmul(
                                pv[:],
                                h_sb[:, 2 * t2:2 * t2 + 2, tb * 128:(tb + 1) * 128],
                                wv_sb[:, 2 * t2:2 * t2 + 2, vc * 512:(vc + 1) * 512],
                                start=(t2 == 0), stop=(t2 == NHT // 2 - 1),
                                perf_mode=DR)
                        nc.scalar.activation(
                            V_sb[:, ci * (CH // 128) + tb, vc * 512:(vc + 1) * 512],
                            pv[:], Copy)

            prev = None
            for ci in range(NCH):
                sl = slice(ci * CH, (ci + 1) * CH)
                x_sb = xin.tile([128, NHT, CH], dt.bfloat16)
                nc.sync.dma_start(out=x_sb[:], in_=xt_t[:, :, sl])
                if ci == 0:
                    # norm weights after x0 so x0 heads the DMA queue
                    nc.sync.dma_start(out=gi_sb[:], in_=gi_t[:, :, 0])
                    nc.sync.dma_start(out=gp_sb[:], in_=gp_t[:, :, 0])
                    # weight loads in halves on the idle Act queue so the
                    # x-chunk loads interleave on the DMA engines
                    wk_t = _rt(wk_e[:])
                    wv_t = _rt(wv_e[:])
                    for hf in range(2):
                        hsl = slice(hf * 512, (hf + 1) * 512)
                        nc.scalar.dma_start(out=wk_sb[:, :, hsl],
                                            in_=wk_t[:, :, hsl])
                        nc.scalar.dma_start(out=wv_sb[:, :, hsl],
                                            in_=wv_t[:, :, hsl])
                if ci == 1:
                    wq_t = _rt(wq_e[:])
                    for hf in range(2):
                        hsl = slice(hf * 512, (hf + 1) * 512)
                        nc.scalar.dma_start(out=wq_sb[:, :, hsl],
                                            in_=wq_t[:, :, hsl])
                    # phase-B x chunk 0 after the A-phase x chunks
                    nc.sync.dma_start(out=xb0_sb[:], in_=xto_t[:, :, 0:QC])
                if ci == 3:
                    # wo for the o-projection, after wk/wv/wq
                    for hf in range(2):
                        hsl = slice(hf * 1024, (hf + 1) * 1024)
                        nc.scalar.dma_start(out=wo_sb[:, :, hsl],
                                            in_=wo_t[:, :, hsl])

                rstd = a_var(ci, x_sb)
                if prev is not None:
                    a_proj(*prev)
                prev = (ci, x_sb, rstd)
            a_proj(*prev)

        mark('B')
        # ============ Phase B: rmsnorm(x_own) -> h8_own; Q^T -> SBUF
        with contextlib.ExitStack() as ph:
            xin2 = ph.enter_context(tc.tile_pool(name="xin2", bufs=2))
            sm2 = ph.enter_context(tc.tile_pool(name="sm2", bufs=3))
            sqp = ph.enter_context(tc.tile_pool(name="sqp2", bufs=10))
            hop = ph.enter_context(tc.tile_pool(name="hop", bufs=2))
            psv = ph.enter_context(tc.tile_pool(name="psv2", bufs=2, space="PSUM"))
            psq = ph.enter_context(tc.tile_pool(name="psq", bufs=2, space="PSUM"))

            def b_var(oc2, xo_sb):
                pvar = psv.tile([1, 256], dt.float32)
                for ht in range(NHT):
                    sq = sqp.tile([128, 256], dt.float32r)
                    if ht % 4 == 0:
                        nc.scalar.activation(sq[:], xo_sb[:, ht, :], Square)
                    else:
                        nc.gpsimd.tensor_mul(sq[:], xo_sb[:, ht, :], xo_sb[:, ht, :])
                    nc.tensor.matmul(pvar[:], ones_r[:], sq[:],
                                     start=(ht == 0), stop=(ht == NHT - 1))
                std = sm2.tile([1, 256], dt.float32)
                nc.scalar.activation(std[:], pvar[:], Sqrt, scale=1.0 / H,
                                     bias=eps1[:])
                rstd = sm2.tile([1, 256], dt.float32r)
                with nc.allow_low_precision(reason="f32r==fp32 bits; PE bcast"):
                    nc.vector.reciprocal(rstd[:], std[:])
                return rstd

            def b_proj(oc2, xo_sb, rstd):
                sl = slice(oc2 * 256, (oc2 + 1) * 256)
                bc = psv.tile([128, 256], dt.float32, tag="bc")
                nc.tensor.matmul(bc[:], ones_col[:], rstd[:],
                                 start=True, stop=True)
                h_own = hop.tile([128, NHT, 256], F8)
                for ht in range(NHT):
                    nc.vector.scalar_tensor_tensor(
                        h_own[:, ht, :], xo_sb[:, ht, :], gi_sb[:, ht:ht + 1],
                        bc[:], MUL, MUL)
                for qc in range(HH // 128):
                    pq = psq.tile([128, 256], dt.float32)
                    for t2 in range(NHT // 2):
                        nc.tensor.matmul(pq[:],
                                         wq_sb[:, 2 * t2:2 * t2 + 2,
                                               qc * 128:(qc + 1) * 128],
                                         h_own[:, 2 * t2:2 * t2 + 2, :],
                                         start=(t2 == 0), stop=(t2 == NHT // 2 - 1),
                                         perf_mode=DR)
                    nc.scalar.activation(Q_sb[:, qc, sl], pq[:], Copy)

            prevb = None
            for oc2 in range(TOK // 256):
                sl = slice(oc2 * 256, (oc2 + 1) * 256)
                if oc2 == 0:
                    xo_sb = xb0_sb
                else:
                    xo_sb = xin2.tile([128, NHT, 256], dt.bfloat16)
                    nc.sync.dma_start(out=xo_sb[:], in_=xto_t[:, :, sl])
                rstd = b_var(oc2, xo_sb)
                if prevb is not None:
                    b_proj(*prevb)
                prevb = (oc2, xo_sb, rstd)
            b_proj(*prevb)

        # ============ Phase C+D: causal attention with the o_proj pair
        # ============ exchange pipelined inside
        r1 = nc.sync.alloc_register("slotr")
        nc.sync.reg_load(r1, slot_e[0:1, 0:1])
        off = nc.sync.snap(r1, donate=True, min_val=0, max_val=1)

        def slot_ap(dram_t, col0, width):
            """Own-slot view with a STATIC dep-tracking offset: writes to
            different column ranges stay independent for the dependency
            tracker instead of forming a serial whole-tensor WAW chain."""
            rt = dram_t[bass.ds(off, 1), :, col0:col0 + width]
            return bass.AP(tensor=rt.tensor, offset=rt.offset, ap=rt.ap,
                           dep_tracking_offset=col0)

        s_c = contextlib.ExitStack()
        mskp = s_c.enter_context(tc.tile_pool(name="mskp", bufs=1))
        msk = mskp.tile([128, 4, QC], dt.float32)
        nc.sync.dma_start(out=msk[:], in_=mk_t[:])

        expp = s_c.enter_context(tc.tile_pool(name="expp", bufs=3))
        esp = s_c.enter_context(tc.tile_pool(name="esp", bufs=2))
        smd = s_c.enter_context(tc.tile_pool(name="smd", bufs=3))
        xoc = s_c.enter_context(tc.tile_pool(name="xoc", bufs=1))
        oop = s_c.enter_context(tc.tile_pool(name="oop", bufs=1))
        rxp = s_c.enter_context(tc.tile_pool(name="rxp", bufs=2))
        sq3p = s_c.enter_context(tc.tile_pool(name="sq3p", bufs=4))
        sm3 = s_c.enter_context(tc.tile_pool(name="sm3", bufs=2))
        pss = s_c.enter_context(tc.tile_pool(name="pss", bufs=1, space="PSUM"))
        pss4 = s_c.enter_context(tc.tile_pool(name="pss4", bufs=1, space="PSUM"))
        psd = s_c.enter_context(tc.tile_pool(name="psd", bufs=1, space="PSUM"))
        psu = s_c.enter_context(tc.tile_pool(name="psu", bufs=1, space="PSUM"))
        pso = s_c.enter_context(tc.tile_pool(name="pso", bufs=2, space="PSUM"))
        psv3 = s_c.enter_context(tc.tile_pool(name="psv3", bufs=1, space="PSUM"))

        ESC = SCALE / (WS * WS)   # fold the x32 Q/K weight scales into exp

        def attn_chunk(oc, at23, after_cc=None):
            # software-pipelined: scores+exp for head h run while head h-1
            # finishes its softmax / attn@V, so the in-order PE never waits
            # on the per-head exp -> denominator chain
            qsl = slice(oc * QC, (oc + 1) * QC)
            lsl = slice((oc % 2) * QC, (oc % 2 + 1) * QC)
            nkb = 4 * (oc + 1)
            nquad = oc             # fully-visible quads (4 key blocks each)
            exps_h = {}

            def scores_exp(h):
                exps = expp.tile([128, NKB, QC], F8, tag="exps")
                exps_h[h] = exps
                for q4 in range(nquad):
                    ps4 = pss4.tile([128, 4, QC], dt.float32)
                    for qq in range(4):
                        kb = 4 * q4 + qq
                        nc.tensor.matmul(ps4[:, qq, :],
                                         K_sb[:, h, kb * 128:(kb + 1) * 128],
                                         Q_sb[:, h, qsl], start=True, stop=True)
                    nc.scalar.activation(exps[:, 4 * q4:4 * q4 + 4, :], ps4[:],
                                         Exp, scale=ESC, bias=biasm2[:])
                for j in range(2 * oc, nkb // 2):
                    ps2 = pss.tile([128, 2, QC], dt.float32)
                    for half in range(2):
                        kb = 2 * j + half
                        nc.tensor.matmul(ps2[:, half, :],
                                         K_sb[:, h, kb * 128:(kb + 1) * 128],
                                         Q_sb[:, h, qsl], start=True, stop=True)
                    d = 2 * j - 4 * oc
                    es2 = esp.tile([128, 2, QC], dt.bfloat16)
                    nc.vector.scalar_tensor_tensor(
                        es2[:], ps2[:], ESC, msk[:, d:d + 2, :], MUL, ADD)
                    nc.scalar.activation(exps[:, 2 * j:2 * j + 2, :], es2[:],
                                         Exp, bias=biasm2[:])

            def softmax_av(h):
                exps = exps_h.pop(h)
                pd = psd.tile([1, QC], dt.float32)
                nc.tensor.matmul(pd[:], eps_ones[:], epsq[:],
                                 start=True, stop=False)
                for kb in range(nkb):
                    nc.tensor.matmul(pd[:], ones_pd[:], exps[:, kb, :],
                                     start=False, stop=(kb == nkb - 1))
                dd = smd.tile([1, QC], dt.float32, tag="dd")
                nc.vector.reciprocal(dd[:], pd[:])
                bcd = smd.tile([128, QC], dt.float32, tag="bcd")
                pb = nc.gpsimd.partition_broadcast(bcd[:], dd[:])
                if after_cc is not None:
                    # keep the Pool queue free for the exchange collective:
                    # this chunk's broadcasts must not be scheduled before it
                    tile.add_dep_helper(pb.ins, after_cc.ins, sync=True,
                                        reason="pbcast after o barrier")
                pu = psu.tile([128, QC], dt.float32)
                for j in range(nkb // 2):
                    nc.tensor.matmul(pu[:],
                                     V_sb[:, 2 * j:2 * j + 2, h * 128:(h + 1) * 128],
                                     exps[:, 2 * j:2 * j + 2, :],
                                     start=(j == 0), stop=(j == nkb // 2 - 1),
                                     perf_mode=DR)
                nc.vector.tensor_tensor(at23[:, h, lsl], pu[:], bcd[:], MUL)

            for h in range(8):
                scores_exp(h)
                if h >= 1:
                    softmax_av(h - 1)
            softmax_av(7)

        def o_chunk(tc_, at23):
            tsl = slice(tc_ * HT, (tc_ + 1) * HT)
            writes = []
            # single fold tile per chunk: per-ocl slices, no slot recycling
            ofold = oop.tile([128, NHT, HT], dt.bfloat16)
            xov = xoc.tile([128, NHT, HT], dt.bfloat16)
            nc.sync.dma_start(out=xov[:], in_=xth_t[:, :, tsl])
            for og in range(4):
                for oi in range(4):
                    ocl = og * 4 + oi
                    po = pso.tile([128, HT], dt.float32)
                    for t2 in range(4):
                        nc.tensor.matmul(
                            po[:],
                            wo_sb[:, 2 * t2:2 * t2 + 2, ocl * 128:(ocl + 1) * 128],
                            at23[:, 2 * t2:2 * t2 + 2, :],
                            start=(t2 == 0), stop=(t2 == 3), perf_mode=DR)
                    # slot partial: po/32 + x/2 (x/2 comes pre-scaled via xtoh)
                    nc.vector.scalar_tensor_tensor(
                        ofold[:, ocl, :], po[:], 1.0 / WS, xov[:, ocl, :],
                        MUL, ADD)
                    d = nc.sync.dma_start(
                        out=slot_ap(xo_d, ocl * TOK + tc_ * HT, HT),
                        in_=ofold[:, ocl, :])
                    writes.append(d)
            return writes

        xo_re = [xo_d[s].rearrange("p (t c) -> p t c", c=TOK) for s in range(2)]

        def x2_chunk(tc_, cc, x2t, rpool):
            # batched 4-tile reads on the HWDGE queue
            for g in range(4):
                gsl = slice(g * 4, (g + 1) * 4)
                csl = slice(tc_ * HT, (tc_ + 1) * HT)
                oa = rpool.tile([128, 4, HT], dt.bfloat16, tag="oa")
                ob = rpool.tile([128, 4, HT], dt.bfloat16, tag="ob")
                da = nc.sync.dma_start(out=oa[:], in_=xo_re[0][:, gsl, csl])
                db = nc.sync.dma_start(out=ob[:], in_=xo_re[1][:, gsl, csl])
                tile.add_dep_helper(da.ins, cc.ins, sync=True,
                                    reason="read after o barrier")
                tile.add_dep_helper(db.ins, cc.ins, sync=True,
                                    reason="read after o barrier")
                nc.vector.tensor_add(x2t[:, gsl, :], oa[:], ob[:])

        def var_chunk(x2t, rstd_out, sqpool, smpool, pvpool):
            pvar3 = pvpool.tile([1, HT], dt.float32, tag="pvar3")
            for ocl in range(NHT):
                sq3 = sqpool.tile([128, HT], dt.float32r)
                nc.vector.tensor_mul(sq3[:], x2t[:, ocl, :], x2t[:, ocl, :])
                nc.tensor.matmul(pvar3[:], ones_r[:], sq3[:],
                                 start=(ocl == 0), stop=(ocl == NHT - 1))
            std3 = smpool.tile([1, HT], dt.float32, tag="std3")
            nc.scalar.activation(std3[:], pvar3[:], Sqrt, scale=1.0 / H,
                                 bias=eps1[:])
            nc.vector.reciprocal(rstd_out[:], std3[:])

        def h2_from(x2t, h2t, rstd, smpool):
            bc2 = smpool.tile([128, HT], dt.float32, tag="bc2")
            nc.gpsimd.partition_broadcast(bc2[:], rstd[:])
            for ocl in range(NHT):
                nc.vector.scalar_tensor_tensor(
                    h2t[:, ocl, :], x2t[:, ocl, :], gp_sb[:, ocl:ocl + 1],
                    bc2[:], MUL, MUL)

        atp_a = contextlib.ExitStack()
        atpool_a = atp_a.enter_context(tc.tile_pool(name="atp_a", bufs=1))
        at23_0 = atpool_a.tile([128, 8, HT], F8)
        mark('attn0')
        attn_chunk(0, at23_0)
        mark('attn1')
        attn_chunk(1, at23_0)
        mark('o0')
        w0 = o_chunk(0, at23_0)
        mark('cc0')
        cc0 = barrier(w0, b1i_d, b1o_d)
        atp_a.close()
        atp_b = s_c.enter_context(tc.tile_pool(name="atp_b", bufs=1))
        at23_1 = atp_b.tile([128, 8, HT], F8)
        mark('attn2')
        attn_chunk(2, at23_1)
        mark('x2_0')
        x2_chunk(0, cc0, x2c0, rxp)
        mark('attn3')
        attn_chunk(3, at23_1, after_cc=cc0)
        mark('o1')
        w1 = o_chunk(1, at23_1)
        mark('cc1')
        cc1 = barrier(w1, b2i_d, b2o_d)
        mark('var0')
        # chunk-0 variance + h2 after o1: their square-trickle overlaps the
        # o-exchange instead of stalling o1's matmuls
        var_chunk(x2c0, rs0, sq3p, sm3, psv3)
        h2_from(x2c0, h2e0, rs0, sm3)

        s_c.close()
        s_kvq.close()
        s_pre.close()

        # ============ Phase E+F: SwiGLU gate/up + down projection, chunk by
        # chunk: pass0 -> down0 -> pass1 -> down1.  aT holds one 512-token
        # chunk (reused); the chunk-1 exchange rides under pass 0, the
        # chunk-1 down barriers interleave with its matmuls so the finalize
        # overlaps the tail.
        x2es = [x2c0, None]
        xd_re = [xd_d[s].rearrange("p (t c) -> p t c", c=TOK) for s in range(2)]
        with contextlib.ExitStack() as ph2:
            atp2 = ph2.enter_context(tc.tile_pool(name="aTp", bufs=1))
            aT = atp2.tile([128, NFT, HT], dt.bfloat16)
            x2p1 = ph2.enter_context(tc.tile_pool(name="x2p1", bufs=1))
            x2c1 = x2p1.tile([128, NHT, HT], dt.bfloat16)
            rs1 = x2p1.tile([1, HT], dt.float32)
            x2es[1] = x2c1
            rxe = ph2.enter_context(tc.tile_pool(name="rxe", bufs=1))
            sq3e = ph2.enter_context(tc.tile_pool(name="sq3e", bufs=2))
            sm3e = ph2.enter_context(tc.tile_pool(name="sm3e", bufs=2))
            h2p1 = ph2.enter_context(tc.tile_pool(name="h2p1", bufs=1))
            h2c1 = h2p1.tile([128, NHT, HT], dt.bfloat16)
            wgp = ph2.enter_context(tc.tile_pool(name="wgp", bufs=2))
            sgp = ph2.enter_context(tc.tile_pool(name="sgp", bufs=2))
            wdp = ph2.enter_context(tc.tile_pool(name="wdp", bufs=3))
            dnp = ph2.enter_context(tc.tile_pool(name="dnp", bufs=2))
            fin = ph2.enter_context(tc.tile_pool(name="fin", bufs=2))
            psg = ph2.enter_context(tc.tile_pool(name="psg", bufs=2, space="PSUM"))
            psn = ph2.enter_context(tc.tile_pool(name="psn", bufs=2, space="PSUM"))
            pvr = ph2.enter_context(tc.tile_pool(name="pvr", bufs=1, space="PSUM"))

            def gateup_pass(tc_, h2t, hooks=()):
                tsl = slice(tc_ * HT, (tc_ + 1) * HT)
                hooks = dict(hooks)
                wtiles = []
                npairs = NFT // 2
                for fp in range(npairs):
                    if fp in hooks:
                        hooks[fp]()
                    eng = nc.gpsimd if fp < 2 else nc.sync
                    hp = (tc.high_priority(offset=400) if fp < 2
                          else contextlib.nullcontext())
                    with hp:
                        wg_sb = wgp.tile([128, NHT, 256], dt.bfloat16, tag="wg")
                        eng.dma_start(out=wg_sb[:],
                                      in_=wg_t[:, :, fp * 256:(fp + 1) * 256])
                        wu_sb = wgp.tile([128, NHT, 256], dt.bfloat16, tag="wu")
                        eng.dma_start(out=wu_sb[:],
                                      in_=wu_t[:, :, fp * 256:(fp + 1) * 256])
                    wtiles.append((wg_sb, wu_sb))
                    if fp >= 1:
                        _gateup_fp(tc_, h2t, tsl, fp - 1, *wtiles[fp - 1])
                _gateup_fp(tc_, h2t, tsl, npairs - 1, *wtiles[-1])

            def _gateup_fp(tc_, h2t, tsl, fp, wg_sb, wu_sb):
                for half in range(2):
                    ff = fp * 2 + half
                    hsl = slice(half * 128, (half + 1) * 128)
                    pg = psg.tile([128, HT], dt.float32, tag="pg")
                    pu = psg.tile([128, HT], dt.float32, tag="pu")
                    for ht in range(NHT):
                        nc.tensor.matmul(pg[:], wg_sb[:, ht, hsl],
                                         h2t[:, ht, :],
                                         start=(ht == 0), stop=(ht == NHT - 1))
                        nc.tensor.matmul(pu[:], wu_sb[:, ht, hsl],
                                         h2t[:, ht, :],
                                         start=(ht == 0), stop=(ht == NHT - 1))
                    sg = sgp.tile([128, HT], dt.float32)
                    nc.scalar.activation(sg[:], pg[:], Silu)
                    nc.vector.tensor_tensor(aT[:, ff, :], sg[:], pu[:], MUL)

            def exchange1():
                # chunk-1 exchange rides under gate/up pass 0
                x2_chunk(1, cc1, x2c1, rxe)
                var_chunk(x2c1, rs1, sq3e, sm3e, pvr)
                h2_from(x2c1, h2c1, rs1, sm3e)

            def _down_hc(chunk, hc, wd_sb, dwrites):
                pn = psn.tile([128, HT], dt.float32)
                for ff in range(NFT):
                    nc.tensor.matmul(pn[:], wd_sb[:, ff, :], aT[:, ff, :],
                                     start=(ff == 0), stop=(ff == NFT - 1))
                # fold x2/2 into the partial
                dn = dnp.tile([128, HT], dt.bfloat16, tag="dn")
                nc.vector.scalar_tensor_tensor(
                    dn[:], x2es[chunk][:, hc, :], 0.5, pn[:], MUL, ADD)
                d = nc.sync.dma_start(
                    out=slot_ap(xd_d, hc * TOK + chunk * HT, HT),
                    in_=dn[:])
                dwrites.append(d)

            def load_wd(hc, eng):
                wd_sb = wdp.tile([128, NFT, 128], dt.bfloat16)
                eng.dma_start(out=wd_sb[:],
                              in_=wd_t[:, :, hc * 128:(hc + 1) * 128])
                return wd_sb

            def down_chunk(chunk, dwrites, bar_at=None, pre_wd=()):
                # bar_at: {n_writes: callable(writes)} -> collectives fired
                # as soon as the n-th slot write has been issued
                bars = []
                wd_tiles = list(pre_wd)

                def _progress():
                    if bar_at and len(dwrites) in bar_at:
                        bars.append(bar_at[len(dwrites)](list(dwrites)))
                        del bar_at[len(dwrites)]

                for hc in range(NHT):
                    if hc < len(pre_wd):
                        if hc >= 2:
                            _down_hc(chunk, hc - 2, wd_tiles[hc - 2], dwrites)
                            _progress()
                        continue
                    eng = nc.gpsimd if hc < 2 else nc.sync
                    hp = (tc.high_priority(offset=500) if hc < 2
                          else contextlib.nullcontext())
                    with hp:
                        wd_sb = load_wd(hc, eng)
                    wd_tiles.append(wd_sb)
                    if hc >= 2:
                        _down_hc(chunk, hc - 2, wd_tiles[hc - 2], dwrites)
                        _progress()
                for hc in (NHT - 2, NHT - 1):
                    _down_hc(chunk, hc, wd_tiles[hc], dwrites)
                    _progress()
                return bars

            def _track(ap, base):
                return bass.AP(tensor=ap.tensor, offset=ap.offset, ap=ap.ap,
                               dep_tracking_offset=base)

            def finalize_own(ccs):
                # own token half only (runtime off), static dep ranges.
                # 4 groups of 4 hc: fewer, bigger DMAs keep the single
                # register-capable (SP) queue short; bf16 output halves the
                # write bytes and makes the adds 2x-rate
                for g in range(4):
                    gsl = slice(g * 4, (g + 1) * 4)
                    gcc = ccs[0] if g < 2 else (ccs[1] if g < 3 else ccs[2])
                    ra = fin.tile([128, 4, HT], dt.bfloat16, tag="ra")
                    rb = fin.tile([128, 4, HT], dt.bfloat16, tag="rb")
                    da = nc.sync.dma_start(
                        out=ra[:],
                        in_=_track(xd_re[0][:, gsl, bass.ds(off * HT, HT)],
                                   4 * g * TOK))
                    db = nc.sync.dma_start(
                        out=rb[:],
                        in_=_track(xd_re[1][:, gsl, bass.ds(off * HT, HT)],
                                   4 * g * TOK))
                    tile.add_dep_helper(da.ins, gcc.ins, sync=True,
                                        reason="read after down barrier")
                    tile.add_dep_helper(db.ins, gcc.ins, sync=True,
                                        reason="read after down barrier")
                    f_t = fin.tile([128, 4, HT], dt.bfloat16, tag="f5")
                    nc.vector.tensor_add(f_t[:], ra[:], rb[:])
                    nc.sync.dma_start(
                        out=_track(out_t[:, gsl, bass.ds(off * HT, HT)],
                                   4 * g * TOK),
                        in_=f_t[:])

            mark('pass0')
            pre_wd0 = []

            def wd_prefetch():
                for hc in range(2):
                    pre_wd0.append(load_wd(hc, nc.gpsimd))

            gateup_pass(0, h2e0, hooks=[(3, exchange1), (14, wd_prefetch)])
            mark('down0')
            dw0 = []
            down_chunk(0, dw0, pre_wd=pre_wd0)
            ccd0 = barrier(dw0, b3i_d, b3o_d)
            mark('pass1')
            gateup_pass(1, h2c1)
            mark('down1')
            dw1 = []
            bars = down_chunk(
                1, dw1,
                bar_at={
                    8: lambda ws: barrier(ws, b3i_d, b3o_d),
                    12: lambda ws: barrier(ws, b4i_d, b4o_d),
                    16: lambda ws: barrier(ws, b1i_d, b1o_d),
                })
            assert len(dw1) == NHT and len(bars) == 3
            mark('finalize')
            finalize_own(bars)

    return nc


_NC_CACHE = None


def _get_nc():
    global _NC_CACHE
    if _NC_CACHE is None:
        _NC_CACHE = build_nc()
        if not _NC_CACHE.is_finalized():
            _NC_CACHE.finalize()
    return _NC_CACHE


def make_in_maps(inputs):
    import ml_dtypes
    bf16 = ml_dtypes.bfloat16
    f8 = ml_dtypes.float8_e4m3fn

    hs = np.asarray(inputs["hidden_states"], dtype=np.float32)
    w = {k: np.asarray(inputs[k], dtype=np.float32) for k in
         ("w_q", "w_k", "w_v", "w_o", "w_gate", "w_up", "w_down")}
    g_in = np.asarray(inputs["g_in"], dtype=np.float32).reshape(H, 1)
    g_post = np.asarray(inputs["g_post"], dtype=np.float32).reshape(H, 1)
    # large finite negative: exp -> 0 exactly, but stays finite in bf16
    # (fp32 min overflows to -inf when the masked scores pass through bf16)
    neg = np.float32(-30000.0)

    in_maps = []
    for c in range(N_CORES):
        p, hh = c // 2, c % 2
        b, par = p // 2, p % 2
        xb = hs[b]                                    # [S, H]
        xt = np.ascontiguousarray(xb.T).astype(bf16)  # [H, S] bf16
        xo = np.ascontiguousarray(xb[par::2].T)       # [H, TOK]
        xt_own = xo.astype(bf16)
        xt_half = (0.5 * xo).astype(bf16)
        # diagonal causal mask blocks: [128 keys, 4 blocks, QC queries]
        k_idx = np.arange(128)[:, None, None]
        d_idx = np.arange(4)[None, :, None]
        q_idx = np.arange(QC)[None, None, :]
        mskd = np.where(d_idx * 128 + k_idx <= 2 * q_idx + par,
                        np.float32(0.0), neg).astype(np.float32)
        mskd = mskd.reshape(128, 4 * QC)
        cs = slice(hh * HH, (hh + 1) * HH)
        fs = slice(hh * FFH, (hh + 1) * FFH)
        in_maps.append({
            "xtb": xt,
            "xtob": xt_own,
            "xtoh": xt_half,
            "mkd": mskd,
            "wq": np.ascontiguousarray(w["w_q"][:, cs] * WS).astype(f8),
            "wk": np.ascontiguousarray(w["w_k"][:, cs] * WS).astype(f8),
            "wv": np.ascontiguousarray(w["w_v"][:, cs] * WS).astype(f8),
            "wo": np.ascontiguousarray(w["w_o"][cs, :] * WS).astype(f8),
            "wg": np.ascontiguousarray(w["w_gate"][:, fs]).astype(bf16),
            "wu": np.ascontiguousarray(w["w_up"][:, fs]).astype(bf16),
            "wd": np.ascontiguousarray(w["w_down"][fs, :]).astype(bf16),
            "g_in": g_in,
            "g_post": g_post,
            "slot": np.array([[hh, 1 - hh]], dtype=np.uint32),
        })
    return in_maps


def assemble_output(results):
    out = np.empty((B, S, H), dtype=np.float32)
    ht = TOK // 2
    for b in range(B):
        for par in range(2):
            c = (2 * b + par) * 2
            pair_out = np.concatenate(
                [results[c]["out"][:, :ht], results[c + 1]["out"][:, ht:]],
                axis=1).astype(np.float32)
            out[b, par::2, :] = pair_out.T
    return out


def kernel(**inputs):
    nc = _get_nc()
    in_maps = make_in_maps(inputs)
    res = run_bass_kernel_spmd(nc, in_maps, list(range(N_CORES)))
    return assemble_output(res.results)


if __name__ == "__main__":
    import time
    t0 = time.time()
    nc = _get_nc()
    print(f"build+finalize: {time.time()-t0:.1f}s")


# revision 30
# speedup vs baseline: 1.0095x; 1.0095x over previous
"""Trainium2 Bass kernel for a dense transformer decoder layer.

Model: B=2, S=2048, H=2048, NH=16, HD=128, FF=8192, fp32 I/O.

Sharding (8 NeuronCores): DP-2 over batch x seq-DP-2 (even/odd token
interleave) across HBM-pairs x TP-2 over heads / FF inside each HBM pair.

  core c: pair p=c//2, head-half hh=c%2; batch b=p//2, parity par=p%2.
  The pair handles the 1024 tokens of batch b at positions par::2.
  Each core owns 8 heads (column half of wq/wk/wv, row half of wo) and
  half of FF.  K/V for all 2048 batch tokens are computed locally
  (replicated inside the batch); the only cross-core traffic is the
  o_proj / down_proj partial-sum exchange between the two cores of an
  HBM pair through pair-shared DRAM, with tiny 2-rank AllGather
  barriers.

v3: the whole attention side (K/V/Q projections, attn@V, o_proj) runs
in fp8e4 with DoubleRow perf mode (2 fp8 k-rows per PE pass = 2x the
bf16 matmul rate).  Weights wq/wk/wv/wo are pre-scaled x32 into fp8 on
the host; the 32x factors cancel through the softmax ratio and fold
into the exp scale / o-fold scalars (o folds x/2 via the pre-halved
xtoh input).  The MLP stays bf16 (fp8 there fails the 2e-2 gate) and
runs pass0 -> down0 -> pass1 -> down1 so aT needs only one 512-token
buffer and the chunk-0 exchange overlaps pass1.  rmsnorm squares ride
the Activation engine (Square), PSUM->SBUF copies ride Activation
(Copy), rstd uses the fused Rsqrt activation, and wq/wo/x-own tiles
prefetch during phase A.  The down-proj chunk-1 barriers interleave
into its matmul stream so the finalize tail overlaps compute.
"""

import sys

sys.path.insert(0, "/opt/trn_rl_repo")

import contextlib

import numpy as np

import concourse.bass as bass
import concourse.tile as tile
from concourse import bacc, mybir
from concourse.bass_utils import run_bass_kernel_spmd

dt = mybir.dt

B, S, H = 2, 2048, 2048
NH, HD = 16, 128
FF = 8192
EPS = 1e-6
N_CORES = 8

TOK = S // 2          # own tokens per pair (1024)
HH = H // 2           # per-core head columns (1024)
FFH = FF // 2         # per-core FF (4096)
NHT = H // 128        # 16
NFT = FFH // 128      # 32
HT = TOK // 2         # 512: exchange chunk
QC = 256              # attention query chunk
NQC = TOK // QC       # 4
NKB = S // 128        # 16 key blocks
WS = 32.0             # host-side fp8 weight scale for wq/wk/wv/wo
SCALE = 1.0 / float(np.sqrt(HD))
PAIRS = [[0, 1], [2, 3], [4, 5], [6, 7]]

Exp = mybir.ActivationFunctionType.Exp
Silu = mybir.ActivationFunctionType.Silu
Sqrt = mybir.ActivationFunctionType.Sqrt
Copy = mybir.ActivationFunctionType.Copy
Square = mybir.ActivationFunctionType.Square
MUL = mybir.AluOpType.mult
ADD = mybir.AluOpType.add
DR = mybir.MatmulPerfMode.DoubleRow
F8 = dt.float8e4


def _rt(ap):
    """[T*128, C] -> [128, T, C] (tile index as middle axis)."""
    return ap.rearrange("(t p) c -> p t c", p=128)


PHASE_MARKS = []


def build_nc():
    nc = bacc.Bacc(None, num_devices=N_CORES)

    def mark(label):
        PHASE_MARKS.append((label, nc.next_id()))

    # ---------------- I/O ----------------
    xt_e = nc.dram_tensor("xtb", [H, S], dt.bfloat16, kind="ExternalInput")
    xto_e = nc.dram_tensor("xtob", [H, TOK], dt.bfloat16, kind="ExternalInput")
    xth_e = nc.dram_tensor("xtoh", [H, TOK], dt.bfloat16, kind="ExternalInput")
    mk_e = nc.dram_tensor("mkd", [128, 4 * QC], dt.float32, kind="ExternalInput")
    wq_e = nc.dram_tensor("wq", [H, HH], F8, kind="ExternalInput")
    wk_e = nc.dram_tensor("wk", [H, HH], F8, kind="ExternalInput")
    wv_e = nc.dram_tensor("wv", [H, HH], F8, kind="ExternalInput")
    wo_e = nc.dram_tensor("wo", [HH, H], F8, kind="ExternalInput")
    wg_e = nc.dram_tensor("wg", [H, FFH], dt.bfloat16, kind="ExternalInput")
    wu_e = nc.dram_tensor("wu", [H, FFH], dt.bfloat16, kind="ExternalInput")
    wd_e = nc.dram_tensor("wd", [FFH, H], dt.bfloat16, kind="ExternalInput")
    gi_e = nc.dram_tensor("g_in", [H, 1], dt.float32, kind="ExternalInput")
    gp_e = nc.dram_tensor("g_post", [H, 1], dt.float32, kind="ExternalInput")
    slot_e = nc.dram_tensor("slot", [1, 2], dt.uint32, kind="ExternalInput")
    out_e = nc.dram_tensor("out", [H, TOK], dt.bfloat16, kind="ExternalOutput")
    # ---------------- internal DRAM ----------------
    xo_d = nc.dram_tensor("xo_d", [2, 128, NHT * TOK], dt.bfloat16, addr_space="Shared")
    xd_d = nc.dram_tensor("xd_d", [2, 128, NHT * TOK], dt.bfloat16, addr_space="Shared")
    b1i_d = nc.dram_tensor("b1i_d", [1, 1], dt.float32)
    b1o_d = nc.dram_tensor("b1o_d", [1, 2], dt.float32)
    b2i_d = nc.dram_tensor("b2i_d", [1, 1], dt.float32)
    b2o_d = nc.dram_tensor("b2o_d", [1, 2], dt.float32)
    b3i_d = nc.dram_tensor("b3i_d", [1, 1], dt.float32)
    b3o_d = nc.dram_tensor("b3o_d", [1, 2], dt.float32)
    b4i_d = nc.dram_tensor("b4i_d", [1, 1], dt.float32)
    b4o_d = nc.dram_tensor("b4o_d", [1, 2], dt.float32)

    xt_t = _rt(xt_e[:])
    xto_t = _rt(xto_e[:])
    xth_t = _rt(xth_e[:])
    mk_t = mk_e[:].rearrange("p (d q) -> p d q", d=4)
    wo_t = _rt(wo_e[:])
    wg_t = _rt(wg_e[:])
    wu_t = _rt(wu_e[:])
    wd_t = _rt(wd_e[:])
    gi_t = _rt(gi_e[:])
    gp_t = _rt(gp_e[:])
    out_t = _rt(out_e[:])

    with tile.TileContext(nc) as tc, contextlib.ExitStack() as top:
        glob = top.enter_context(tc.tile_pool(name="glob", bufs=1))
        ones_r = glob.tile([128, 1], dt.float32r)
        ones_pd = glob.tile([128, 1], F8)
        tmp1 = glob.tile([128, 1], dt.float32)
        nc.vector.memset(tmp1[:], 1.0)
        nc.vector.tensor_copy(ones_r[:], tmp1[:])
        tmp2 = glob.tile([128, 1], dt.float32)
        nc.vector.memset(tmp2[:], WS)
        nc.vector.tensor_copy(ones_pd[:], tmp2[:])
        eps1 = glob.tile([1, 1], dt.float32)
        nc.vector.memset(eps1[:], EPS)
        biasm2 = glob.tile([128, 1], dt.float32)
        nc.vector.memset(biasm2[:], -2.0)
        eps_ones = glob.tile([128, 1], F8)
        epsq = glob.tile([128, QC], F8)
        nc.vector.memset(tmp2[:], 2.0 ** -9)
        nc.vector.tensor_copy(eps_ones[:], tmp2[:])
        nc.vector.memset(epsq[:], 2.0 ** -9)
        ones_col = glob.tile([1, 128], dt.float32r)
        tmp5 = glob.tile([1, 128], dt.float32)
        nc.vector.memset(tmp5[:], 1.0)
        nc.vector.tensor_copy(ones_col[:], tmp5[:])
        flag = glob.tile([1, 1], dt.float32)
        nc.vector.memset(flag[:], 1.0)
        gi_sb = glob.tile([128, NHT], dt.float32)
        gp_sb = glob.tile([128, NHT], dt.float32)

        def barrier(writes, bi_d, bo_d):
            nc.sync.dma_start(out=bi_d[:], in_=flag[:])
            cc = nc.gpsimd.collective_compute(
                "AllGather", mybir.AluOpType.bypass, replica_groups=PAIRS,
                ins=[bi_d[:].opt()], outs=[bo_d[:].opt()])
            for d in writes:
                tile.add_dep_helper(cc.ins, d.ins, sync=True,
                                    reason="partial writes before barrier")
            return cc

        # chunk-0 x2 (residual), its rstd and its h2 stay SBUF-resident
        # across the whole kernel (outer pool: survives the kvq release)
        x2p0 = top.enter_context(tc.tile_pool(name="x2p0", bufs=1))
        x2c0 = x2p0.tile([128, NHT, HT], dt.bfloat16)
        rs0 = x2p0.tile([1, HT], dt.float32)
        h2e0 = x2p0.tile([128, NHT, HT], dt.bfloat16)

        # prefetch tiles that die with the attention phase: wq/wo (fp8) and
        # the phase-B chunk-0 x tile
        s_pre = contextlib.ExitStack()
        pre = s_pre.enter_context(tc.tile_pool(name="pre", bufs=1))
        wq_sb = pre.tile([128, NHT, HH], F8)
        wo_sb = pre.tile([128, 8, H], F8)
        xb0_sb = pre.tile([128, NHT, QC], dt.bfloat16)

        # attention activations: freed after phase C so the MLP can use
        # the space (pool releases must nest, hence the dedicated stack)
        s_kvq = contextlib.ExitStack()
        kvq = s_kvq.enter_context(tc.tile_pool(name="kvq", bufs=1))
        K_sb = kvq.tile([128, 8, S], F8)        # K^T (x32) per own head
        V_sb = kvq.tile([128, NKB, HH], F8)     # V (x32), token-part blocks
        Q_sb = kvq.tile([128, 8, TOK], F8)      # Q^T (x32) per own head

        mark('A')
        # ============ Phase A: rmsnorm(x) -> h8; K^T and V for all S tokens
        CH = 256
        NCH = S // CH
        with contextlib.ExitStack() as ph:
            wkv = ph.enter_context(tc.tile_pool(name="wkv", bufs=1))
            xin = ph.enter_context(tc.tile_pool(name="xin", bufs=2))
            hpool = ph.enter_context(tc.tile_pool(name="hpool", bufs=2))
            sm1 = ph.enter_context(tc.tile_pool(name="sm1", bufs=3))
            sqp = ph.enter_context(tc.tile_pool(name="sqp", bufs=10))
            psv = ph.enter_context(tc.tile_pool(name="psv", bufs=2, space="PSUM"))
            psk = ph.enter_context(tc.tile_pool(name="psk", bufs=2, space="PSUM"))

            wk_sb = wkv.tile([128, NHT, HH], F8)
            wv_sb = wkv.tile([128, NHT, HH], F8)

            def a_var(ci, x_sb):
                pvar = psv.tile([1, CH], dt.float32)
                for ht in range(NHT):
                    sq = sqp.tile([128, CH], dt.float32r)
                    if ht % 4 == 1:
                        nc.vector.tensor_mul(sq[:], x_sb[:, ht, :], x_sb[:, ht, :])
                    else:
                        nc.gpsimd.tensor_mul(sq[:], x_sb[:, ht, :], x_sb[:, ht, :])
                    nc.tensor.matmul(pvar[:], ones_r[:], sq[:],
                                     start=(ht == 0), stop=(ht == NHT - 1))
                std = sm1.tile([1, CH], dt.float32)
                nc.scalar.activation(std[:], pvar[:], Sqrt, scale=1.0 / H,
                                     bias=eps1[:])
                rstd = sm1.tile([1, CH], dt.float32r)
                with nc.allow_low_precision(reason="f32r==fp32 bits; PE bcast"):
                    nc.vector.reciprocal(rstd[:], std[:])
                return rstd

            def a_proj(ci, x_sb, rstd):
                sl = slice(ci * CH, (ci + 1) * CH)
                # partition-broadcast rstd on the PE (K=1 matmul): keeps the
                # Pool queue free of the variance chain so squares pipeline.
                # Two stacked copies let h8 run in ht-pairs (half the DVE
                # per-op PSUM penalty); the rmsnorm gain is folded into the
                # wq/wk/wv weights on the host.
                bc = psv.tile([128, 2, CH], dt.float32, tag="bc")
                for i in range(2):
                    nc.tensor.matmul(bc[:, i, :], ones_col[:], rstd[:],
                                     start=True, stop=True)
                h_sb = hpool.tile([128, NHT, CH], F8)
                for t2 in range(NHT // 2):
                    nc.vector.tensor_mul(h_sb[:, 2 * t2:2 * t2 + 2, :],
                                         x_sb[:, 2 * t2:2 * t2 + 2, :], bc[:])
                # K^T tiles [kcol 128, CH] -> K_sb (fp8 DoubleRow)
                for kc in range(HH // 128):
                    pk = psk.tile([128, CH], dt.float32, tag="pk")
                    for t2 in range(NHT // 2):
                        nc.tensor.matmul(pk[:],
                                         wk_sb[:, 2 * t2:2 * t2 + 2,
                                               kc * 128:(kc + 1) * 128],
                                         h_sb[:, 2 * t2:2 * t2 + 2, :],
                                         start=(t2 == 0), stop=(t2 == NHT // 2 - 1),
                                         perf_mode=DR)
                    nc.scalar.activation(K_sb[:, kc, sl], pk[:], Copy)
                # V tiles [tok 128, 512] -> V_sb (fp8 DoubleRow)
                for tb in range(CH // 128):
                    for vc in range(HH // 512):
                        pv = psk.tile([128, 512], dt.float32, tag="pv")
                        for t2 in range(NHT // 2):
                            nc.tensor.matmul(
                                pv[:],
                                h_sb[:, 2 * t2:2 * t2 + 2, tb * 128:(tb + 1) * 128],
                                wv_sb[:, 2 * t2:2 * t2 + 2, vc * 512:(vc + 1) * 512],
                                start=(t2 == 0), stop=(t2 == NHT // 2 - 1),
                                perf_mode=DR)
                        nc.scalar.activation(
                            V_sb[:, ci * (CH // 128) + tb, vc * 512:(vc + 1) * 512],
                            pv[:], Copy)

            prev = None
            for ci in range(NCH):
                sl = slice(ci * CH, (ci + 1) * CH)
                x_sb = xin.tile([128, NHT, CH], dt.bfloat16)
                nc.sync.dma_start(out=x_sb[:], in_=xt_t[:, :, sl])
                if ci == 0:
                    # norm weights after x0 so x0 heads the DMA queue
                    nc.sync.dma_start(out=gi_sb[:], in_=gi_t[:, :, 0])
                    nc.sync.dma_start(out=gp_sb[:], in_=gp_t[:, :, 0])
                    # weight loads in halves on the idle Act queue so the
                    # x-chunk loads interleave on the DMA engines
                    wk_t = _rt(wk_e[:])
                    wv_t = _rt(wv_e[:])
                    for hf in range(2):
                        hsl = slice(hf * 512, (hf + 1) * 512)
                        nc.scalar.dma_start(out=wk_sb[:, :, hsl],
                                            in_=wk_t[:, :, hsl])
                        nc.scalar.dma_start(out=wv_sb[:, :, hsl],
                                            in_=wv_t[:, :, hsl])
                if ci == 1:
                    wq_t = _rt(wq_e[:])
                    for hf in range(2):
                        hsl = slice(hf * 512, (hf + 1) * 512)
                        nc.scalar.dma_start(out=wq_sb[:, :, hsl],
                                            in_=wq_t[:, :, hsl])
                    # phase-B x chunk 0 after the A-phase x chunks
                    nc.sync.dma_start(out=xb0_sb[:], in_=xto_t[:, :, 0:QC])
                if ci == 3:
                    # wo for the o-projection, after wk/wv/wq
                    for hf in range(2):
                        hsl = slice(hf * 1024, (hf + 1) * 1024)
                        nc.scalar.dma_start(out=wo_sb[:, :, hsl],
                                            in_=wo_t[:, :, hsl])

                rstd = a_var(ci, x_sb)
                if prev is not None:
                    a_proj(*prev)
                prev = (ci, x_sb, rstd)
            a_proj(*prev)

        mark('B')
        # ============ Phase B: rmsnorm(x_own) -> h8_own; Q^T -> SBUF
        with contextlib.ExitStack() as ph:
            xin2 = ph.enter_context(tc.tile_pool(name="xin2", bufs=2))
            sm2 = ph.enter_context(tc.tile_pool(name="sm2", bufs=3))
            sqp = ph.enter_context(tc.tile_pool(name="sqp2", bufs=10))
            hop = ph.enter_context(tc.tile_pool(name="hop", bufs=2))
            psv = ph.enter_context(tc.tile_pool(name="psv2", bufs=2, space="PSUM"))
            psq = ph.enter_context(tc.tile_pool(name="psq", bufs=2, space="PSUM"))

            def b_var(oc2, xo_sb):
                pvar = psv.tile([1, 256], dt.float32)
                for ht in range(NHT):
                    sq = sqp.tile([128, 256], dt.float32r)
                    if ht % 4 == 0:
                        nc.scalar.activation(sq[:], xo_sb[:, ht, :], Square)
                    else:
                        nc.gpsimd.tensor_mul(sq[:], xo_sb[:, ht, :], xo_sb[:, ht, :])
                    nc.tensor.matmul(pvar[:], ones_r[:], sq[:],
                                     start=(ht == 0), stop=(ht == NHT - 1))
                std = sm2.tile([1, 256], dt.float32)
                nc.scalar.activation(std[:], pvar[:], Sqrt, scale=1.0 / H,
                                     bias=eps1[:])
                rstd = sm2.tile([1, 256], dt.float32r)
                with nc.allow_low_precision(reason="f32r==fp32 bits; PE bcast"):
                    nc.vector.reciprocal(rstd[:], std[:])
                return rstd

            def b_proj(oc2, xo_sb, rstd):
                sl = slice(oc2 * 256, (oc2 + 1) * 256)
                bc = psv.tile([128, 2, 256], dt.float32, tag="bc")
                for i in range(2):
                    nc.tensor.matmul(bc[:, i, :], ones_col[:], rstd[:],
                                     start=True, stop=True)
                h_own = hop.tile([128, NHT, 256], F8)
                for t2 in range(NHT // 2):
                    nc.vector.tensor_mul(h_own[:, 2 * t2:2 * t2 + 2, :],
                                         xo_sb[:, 2 * t2:2 * t2 + 2, :], bc[:])
                for qc in range(HH // 128):
                    pq = psq.tile([128, 256], dt.float32)
                    for t2 in range(NHT // 2):
                        nc.tensor.matmul(pq[:],
                                         wq_sb[:, 2 * t2:2 * t2 + 2,
                                               qc * 128:(qc + 1) * 128],
                                         h_own[:, 2 * t2:2 * t2 + 2, :],
                                         start=(t2 == 0), stop=(t2 == NHT // 2 - 1),
                                         perf_mode=DR)
                    nc.scalar.activation(Q_sb[:, qc, sl], pq[:], Copy)

            prevb = None
            for oc2 in range(TOK // 256):
                sl = slice(oc2 * 256, (oc2 + 1) * 256)
                if oc2 == 0:
                    xo_sb = xb0_sb
                else:
                    xo_sb = xin2.tile([128, NHT, 256], dt.bfloat16)
                    nc.sync.dma_start(out=xo_sb[:], in_=xto_t[:, :, sl])
                rstd = b_var(oc2, xo_sb)
                if prevb is not None:
                    b_proj(*prevb)
                prevb = (oc2, xo_sb, rstd)
            b_proj(*prevb)

        # ============ Phase C+D: causal attention with the o_proj pair
        # ============ exchange pipelined inside
        r1 = nc.sync.alloc_register("slotr")
        nc.sync.reg_load(r1, slot_e[0:1, 0:1])
        off = nc.sync.snap(r1, donate=True, min_val=0, max_val=1)

        def slot_ap(dram_t, col0, width):
            """Own-slot view with a STATIC dep-tracking offset: writes to
            different column ranges stay independent for the dependency
            tracker instead of forming a serial whole-tensor WAW chain."""
            rt = dram_t[bass.ds(off, 1), :, col0:col0 + width]
            return bass.AP(tensor=rt.tensor, offset=rt.offset, ap=rt.ap,
                           dep_tracking_offset=col0)

        s_c = contextlib.ExitStack()
        mskp = s_c.enter_context(tc.tile_pool(name="mskp", bufs=1))
        msk = mskp.tile([128, 4, QC], dt.float32)
        nc.sync.dma_start(out=msk[:], in_=mk_t[:])

        expp = s_c.enter_context(tc.tile_pool(name="expp", bufs=3))
        esp = s_c.enter_context(tc.tile_pool(name="esp", bufs=2))
        smd = s_c.enter_context(tc.tile_pool(name="smd", bufs=3))
        xoc = s_c.enter_context(tc.tile_pool(name="xoc", bufs=1))
        oop = s_c.enter_context(tc.tile_pool(name="oop", bufs=1))
        rxp = s_c.enter_context(tc.tile_pool(name="rxp", bufs=2))
        sq3p = s_c.enter_context(tc.tile_pool(name="sq3p", bufs=4))
        sm3 = s_c.enter_context(tc.tile_pool(name="sm3", bufs=2))
        pss = s_c.enter_context(tc.tile_pool(name="pss", bufs=1, space="PSUM"))
        pss4 = s_c.enter_context(tc.tile_pool(name="pss4", bufs=1, space="PSUM"))
        psd = s_c.enter_context(tc.tile_pool(name="psd", bufs=1, space="PSUM"))
        psu = s_c.enter_context(tc.tile_pool(name="psu", bufs=1, space="PSUM"))
        pso = s_c.enter_context(tc.tile_pool(name="pso", bufs=2, space="PSUM"))
        psv3 = s_c.enter_context(tc.tile_pool(name="psv3", bufs=1, space="PSUM"))

        ESC = SCALE / (WS * WS)   # fold the x32 Q/K weight scales into exp

        def attn_chunk(oc, at23, after_cc=None):
            # software-pipelined: scores+exp for head h run while head h-1
            # finishes its softmax / attn@V, so the in-order PE never waits
            # on the per-head exp -> denominator chain
            qsl = slice(oc * QC, (oc + 1) * QC)
            lsl = slice((oc % 2) * QC, (oc % 2 + 1) * QC)
            nkb = 4 * (oc + 1)
            nquad = oc             # fully-visible quads (4 key blocks each)
            exps_h = {}

            def scores_exp(h):
                exps = expp.tile([128, NKB, QC], F8, tag="exps")
                exps_h[h] = exps
                for q4 in range(nquad):
                    ps4 = pss4.tile([128, 4, QC], dt.float32)
                    for qq in range(4):
                        kb = 4 * q4 + qq
                        nc.tensor.matmul(ps4[:, qq, :],
                                         K_sb[:, h, kb * 128:(kb + 1) * 128],
                                         Q_sb[:, h, qsl], start=True, stop=True)
                    nc.scalar.activation(exps[:, 4 * q4:4 * q4 + 4, :], ps4[:],
                                         Exp, scale=ESC, bias=biasm2[:])
                for j in range(2 * oc, nkb // 2):
                    ps2 = pss.tile([128, 2, QC], dt.float32)
                    for half in range(2):
                        kb = 2 * j + half
                        nc.tensor.matmul(ps2[:, half, :],
                                         K_sb[:, h, kb * 128:(kb + 1) * 128],
                                         Q_sb[:, h, qsl], start=True, stop=True)
                    d = 2 * j - 4 * oc
                    es2 = esp.tile([128, 2, QC], dt.bfloat16)
                    nc.vector.scalar_tensor_tensor(
                        es2[:], ps2[:], ESC, msk[:, d:d + 2, :], MUL, ADD)
                    nc.scalar.activation(exps[:, 2 * j:2 * j + 2, :], es2[:],
                                         Exp, bias=biasm2[:])

            def softmax_av(h):
                exps = exps_h.pop(h)
                pd = psd.tile([1, QC], dt.float32)
                nc.tensor.matmul(pd[:], eps_ones[:], epsq[:],
                                 start=True, stop=False)
                for kb in range(nkb):
                    nc.tensor.matmul(pd[:], ones_pd[:], exps[:, kb, :],
                                     start=False, stop=(kb == nkb - 1))
                dd = smd.tile([1, QC], dt.float32, tag="dd")
                nc.vector.reciprocal(dd[:], pd[:])
                bcd = smd.tile([128, QC], dt.float32, tag="bcd")
                pb = nc.gpsimd.partition_broadcast(bcd[:], dd[:])
                if after_cc is not None:
                    # keep the Pool queue free for the exchange collective:
                    # this chunk's broadcasts must not be scheduled before it
                    tile.add_dep_helper(pb.ins, after_cc.ins, sync=True,
                                        reason="pbcast after o barrier")
                pu = psu.tile([128, QC], dt.float32)
                for j in range(nkb // 2):
                    nc.tensor.matmul(pu[:],
                                     V_sb[:, 2 * j:2 * j + 2, h * 128:(h + 1) * 128],
                                     exps[:, 2 * j:2 * j + 2, :],
                                     start=(j == 0), stop=(j == nkb // 2 - 1),
                                     perf_mode=DR)
                nc.vector.tensor_tensor(at23[:, h, lsl], pu[:], bcd[:], MUL)

            for h in range(8):
                scores_exp(h)
                if h >= 1:
                    softmax_av(h - 1)
            softmax_av(7)

        def o_chunk(tc_, at23):
            tsl = slice(tc_ * HT, (tc_ + 1) * HT)
            writes = []
            # single fold tile per chunk: per-ocl slices, no slot recycling
            ofold = oop.tile([128, NHT, HT], dt.bfloat16)
            xov = xoc.tile([128, NHT, HT], dt.bfloat16)
            nc.sync.dma_start(out=xov[:], in_=xth_t[:, :, tsl])
            for og in range(4):
                for oi in range(4):
                    ocl = og * 4 + oi
                    po = pso.tile([128, HT], dt.float32)
                    for t2 in range(4):
                        nc.tensor.matmul(
                            po[:],
                            wo_sb[:, 2 * t2:2 * t2 + 2, ocl * 128:(ocl + 1) * 128],
                            at23[:, 2 * t2:2 * t2 + 2, :],
                            start=(t2 == 0), stop=(t2 == 3), perf_mode=DR)
                    # slot partial: po/32 + x/2 (x/2 comes pre-scaled via xtoh)
                    nc.vector.scalar_tensor_tensor(
                        ofold[:, ocl, :], po[:], 1.0 / WS, xov[:, ocl, :],
                        MUL, ADD)
                    d = nc.sync.dma_start(
                        out=slot_ap(xo_d, ocl * TOK + tc_ * HT, HT),
                        in_=ofold[:, ocl, :])
                    writes.append(d)
            return writes

        xo_re = [xo_d[s].rearrange("p (t c) -> p t c", c=TOK) for s in range(2)]

        def x2_chunk(tc_, cc, x2t, rpool):
            # batched 4-tile reads on the HWDGE queue
            for g in range(4):
                gsl = slice(g * 4, (g + 1) * 4)
                csl = slice(tc_ * HT, (tc_ + 1) * HT)
                oa = rpool.tile([128, 4, HT], dt.bfloat16, tag="oa")
                ob = rpool.tile([128, 4, HT], dt.bfloat16, tag="ob")
                da = nc.sync.dma_start(out=oa[:], in_=xo_re[0][:, gsl, csl])
                db = nc.sync.dma_start(out=ob[:], in_=xo_re[1][:, gsl, csl])
                tile.add_dep_helper(da.ins, cc.ins, sync=True,
                                    reason="read after o barrier")
                tile.add_dep_helper(db.ins, cc.ins, sync=True,
                                    reason="read after o barrier")
                nc.vector.tensor_add(x2t[:, gsl, :], oa[:], ob[:])

        def var_chunk(x2t, rstd_out, sqpool, smpool, pvpool):
            pvar3 = pvpool.tile([1, HT], dt.float32, tag="pvar3")
            for ocl in range(NHT):
                sq3 = sqpool.tile([128, HT], dt.float32r)
                nc.vector.tensor_mul(sq3[:], x2t[:, ocl, :], x2t[:, ocl, :])
                nc.tensor.matmul(pvar3[:], ones_r[:], sq3[:],
                                 start=(ocl == 0), stop=(ocl == NHT - 1))
            std3 = smpool.tile([1, HT], dt.float32, tag="std3")
            nc.scalar.activation(std3[:], pvar3[:], Sqrt, scale=1.0 / H,
                                 bias=eps1[:])
            nc.vector.reciprocal(rstd_out[:], std3[:])

        def h2_from(x2t, h2t, rstd, smpool):
            bc2 = smpool.tile([128, HT], dt.float32, tag="bc2")
            nc.gpsimd.partition_broadcast(bc2[:], rstd[:])
            for ocl in range(NHT):
                nc.vector.scalar_tensor_tensor(
                    h2t[:, ocl, :], x2t[:, ocl, :], gp_sb[:, ocl:ocl + 1],
                    bc2[:], MUL, MUL)

        atp_a = contextlib.ExitStack()
        atpool_a = atp_a.enter_context(tc.tile_pool(name="atp_a", bufs=1))
        at23_0 = atpool_a.tile([128, 8, HT], F8)
        mark('attn0')
        attn_chunk(0, at23_0)
        mark('attn1')
        attn_chunk(1, at23_0)
        mark('o0')
        w0 = o_chunk(0, at23_0)
        mark('cc0')
        cc0 = barrier(w0, b1i_d, b1o_d)
        atp_a.close()
        atp_b = s_c.enter_context(tc.tile_pool(name="atp_b", bufs=1))
        at23_1 = atp_b.tile([128, 8, HT], F8)
        mark('attn2')
        attn_chunk(2, at23_1)
        mark('x2_0')
        x2_chunk(0, cc0, x2c0, rxp)
        mark('attn3')
        attn_chunk(3, at23_1, after_cc=cc0)
        mark('o1')
        w1 = o_chunk(1, at23_1)
        mark('cc1')
        cc1 = barrier(w1, b2i_d, b2o_d)
        mark('var0')
        # chunk-0 variance + h2 after o1: their square-trickle overlaps the
        # o-exchange instead of stalling o1's matmuls
        var_chunk(x2c0, rs0, sq3p, sm3, psv3)
        h2_from(x2c0, h2e0, rs0, sm3)

        s_c.close()
        s_kvq.close()
        s_pre.close()

        # ============ Phase E+F: SwiGLU gate/up + down projection, chunk by
        # chunk: pass0 -> down0 -> pass1 -> down1.  aT holds one 512-token
        # chunk (reused); the chunk-1 exchange rides under pass 0, the
        # chunk-1 down barriers interleave with its matmuls so the finalize
        # overlaps the tail.
        x2es = [x2c0, None]
        xd_re = [xd_d[s].rearrange("p (t c) -> p t c", c=TOK) for s in range(2)]
        with contextlib.ExitStack() as ph2:
            atp2 = ph2.enter_context(tc.tile_pool(name="aTp", bufs=1))
            aT = atp2.tile([128, NFT, HT], dt.bfloat16)
            x2p1 = ph2.enter_context(tc.tile_pool(name="x2p1", bufs=1))
            x2c1 = x2p1.tile([128, NHT, HT], dt.bfloat16)
            rs1 = x2p1.tile([1, HT], dt.float32)
            x2es[1] = x2c1
            rxe = ph2.enter_context(tc.tile_pool(name="rxe", bufs=1))
            sq3e = ph2.enter_context(tc.tile_pool(name="sq3e", bufs=2))
            sm3e = ph2.enter_context(tc.tile_pool(name="sm3e", bufs=2))
            h2p1 = ph2.enter_context(tc.tile_pool(name="h2p1", bufs=1))
            h2c1 = h2p1.tile([128, NHT, HT], dt.bfloat16)
            wgp = ph2.enter_context(tc.tile_pool(name="wgp", bufs=2))
            sgp = ph2.enter_context(tc.tile_pool(name="sgp", bufs=2))
            wdp = ph2.enter_context(tc.tile_pool(name="wdp", bufs=3))
            dnp = ph2.enter_context(tc.tile_pool(name="dnp", bufs=2))
            fin = ph2.enter_context(tc.tile_pool(name="fin", bufs=2))
            psg = ph2.enter_context(tc.tile_pool(name="psg", bufs=2, space="PSUM"))
            psn = ph2.enter_context(tc.tile_pool(name="psn", bufs=2, space="PSUM"))
            pvr = ph2.enter_context(tc.tile_pool(name="pvr", bufs=1, space="PSUM"))

            def gateup_pass(tc_, h2t, hooks=()):
                tsl = slice(tc_ * HT, (tc_ + 1) * HT)
                hooks = dict(hooks)
                wtiles = []
                npairs = NFT // 2
                for fp in range(npairs):
                    if fp in hooks:
                        hooks[fp]()
                    eng = nc.gpsimd if fp < 2 else nc.sync
                    hp = (tc.high_priority(offset=400) if fp < 2
                          else contextlib.nullcontext())
                    with hp:
                        wg_sb = wgp.tile([128, NHT, 256], dt.bfloat16, tag="wg")
                        eng.dma_start(out=wg_sb[:],
                                      in_=wg_t[:, :, fp * 256:(fp + 1) * 256])
                        wu_sb = wgp.tile([128, NHT, 256], dt.bfloat16, tag="wu")
                        eng.dma_start(out=wu_sb[:],
                                      in_=wu_t[:, :, fp * 256:(fp + 1) * 256])
                    wtiles.append((wg_sb, wu_sb))
                    if fp >= 1:
                        _gateup_fp(tc_, h2t, tsl, fp - 1, *wtiles[fp - 1])
                _gateup_fp(tc_, h2t, tsl, npairs - 1, *wtiles[-1])

            def _gateup_fp(tc_, h2t, tsl, fp, wg_sb, wu_sb):
                for half in range(2):
                    ff = fp * 2 + half
                    hsl = slice(half * 128, (half + 1) * 128)
                    pg = psg.tile([128, HT], dt.float32, tag="pg")
                    pu = psg.tile([128, HT], dt.float32, tag="pu")
                    for ht in range(NHT):
                        nc.tensor.matmul(pg[:], wg_sb[:, ht, hsl],
                                         h2t[:, ht, :],
                                         start=(ht == 0), stop=(ht == NHT - 1))
                        nc.tensor.matmul(pu[:], wu_sb[:, ht, hsl],
                                         h2t[:, ht, :],
                                         start=(ht == 0), stop=(ht == NHT - 1))
                    sg = sgp.tile([128, HT], dt.float32)
                    nc.scalar.activation(sg[:], pg[:], Silu)
                    nc.vector.tensor_tensor(aT[:, ff, :], sg[:], pu[:], MUL)

            def exchange1():
                # chunk-1 exchange rides under gate/up pass 0
                x2_chunk(1, cc1, x2c1, rxe)
                var_chunk(x2c1, rs1, sq3e, sm3e, pvr)
                h2_from(x2c1, h2c1, rs1, sm3e)

            def _down_hc(chunk, hc, wd_sb, dwrites):
                pn = psn.tile([128, HT], dt.float32)
                for ff in range(NFT):
                    nc.tensor.matmul(pn[:], wd_sb[:, ff, :], aT[:, ff, :],
                                     start=(ff == 0), stop=(ff == NFT - 1))
                # fold x2/2 into the partial
                dn = dnp.tile([128, HT], dt.bfloat16, tag="dn")
                nc.vector.scalar_tensor_tensor(
                    dn[:], x2es[chunk][:, hc, :], 0.5, pn[:], MUL, ADD)
                d = nc.sync.dma_start(
                    out=slot_ap(xd_d, hc * TOK + chunk * HT, HT),
                    in_=dn[:])
                dwrites.append(d)

            def load_wd(hc, eng):
                wd_sb = wdp.tile([128, NFT, 128], dt.bfloat16)
                eng.dma_start(out=wd_sb[:],
                              in_=wd_t[:, :, hc * 128:(hc + 1) * 128])
                return wd_sb

            def down_chunk(chunk, dwrites, bar_at=None, pre_wd=()):
                # bar_at: {n_writes: callable(writes)} -> collectives fired
                # as soon as the n-th slot write has been issued
                bars = []
                wd_tiles = list(pre_wd)

                def _progress():
                    if bar_at and len(dwrites) in bar_at:
                        bars.append(bar_at[len(dwrites)](list(dwrites)))
                        del bar_at[len(dwrites)]

                for hc in range(NHT):
                    if hc < len(pre_wd):
                        if hc >= 2:
                            _down_hc(chunk, hc - 2, wd_tiles[hc - 2], dwrites)
                            _progress()
                        continue
                    eng = nc.gpsimd if hc < 2 else nc.sync
                    hp = (tc.high_priority(offset=500) if hc < 2
                          else contextlib.nullcontext())
                    with hp:
                        wd_sb = load_wd(hc, eng)
                    wd_tiles.append(wd_sb)
                    if hc >= 2:
                        _down_hc(chunk, hc - 2, wd_tiles[hc - 2], dwrites)
                        _progress()
                for hc in (NHT - 2, NHT - 1):
                    _down_hc(chunk, hc, wd_tiles[hc], dwrites)
                    _progress()
                return bars

            def _track(ap, base):
                return bass.AP(tensor=ap.tensor, offset=ap.offset, ap=ap.ap,
                               dep_tracking_offset=base)

            def finalize_own(ccs):
                # own token half only (runtime off), static dep ranges.
                # 4 groups of 4 hc: fewer, bigger DMAs keep the single
                # register-capable (SP) queue short; bf16 output halves the
                # write bytes and makes the adds 2x-rate
                for g in range(4):
                    gsl = slice(g * 4, (g + 1) * 4)
                    gcc = ccs[0] if g < 2 else (ccs[1] if g < 3 else ccs[2])
                    ra = fin.tile([128, 4, HT], dt.bfloat16, tag="ra")
                    rb = fin.tile([128, 4, HT], dt.bfloat16, tag="rb")
                    da = nc.sync.dma_start(
                        out=ra[:],
                        in_=_track(xd_re[0][:, gsl, bass.ds(off * HT, HT)],
                                   4 * g * TOK))
                    db = nc.sync.dma_start(
                        out=rb[:],
                        in_=_track(xd_re[1][:, gsl, bass.ds(off * HT, HT)],
                                   4 * g * TOK))
                    tile.add_dep_helper(da.ins, gcc.ins, sync=True,
                                        reason="read after down barrier")
                    tile.add_dep_helper(db.ins, gcc.ins, sync=True,
                                        reason="read after down barrier")
                    f_t = fin.tile([128, 4, HT], dt.bfloat16, tag="f5")
                    nc.vector.tensor_add(f_t[:], ra[:], rb[:])
                    nc.sync.dma_start(
                        out=_track(out_t[:, gsl, bass.ds(off * HT, HT)],
                                   4 * g * TOK),
                        in_=f_t[:])

            mark('pass0')
            pre_wd0 = []

            def wd_prefetch():
                for hc in range(2):
                    pre_wd0.append(load_wd(hc, nc.gpsimd))

            gateup_pass(0, h2e0, hooks=[(3, exchange1), (14, wd_prefetch)])
            mark('down0')
            dw0 = []
            down_chunk(0, dw0, pre_wd=pre_wd0)
            ccd0 = barrier(dw0, b3i_d, b3o_d)
            mark('pass1')
            gateup_pass(1, h2c1)
            mark('down1')
            dw1 = []
            bars = down_chunk(
                1, dw1,
                bar_at={
                    8: lambda ws: barrier(ws, b3i_d, b3o_d),
                    12: lambda ws: barrier(ws, b4i_d, b4o_d),
                    16: lambda ws: barrier(ws, b1i_d, b1o_d),
                })
            assert len(dw1) == NHT and len(bars) == 3
            mark('finalize')
            finalize_own(bars)

    return nc


_NC_CACHE = None


def _get_nc():
    global _NC_CACHE
    if _NC_CACHE is None:
        _NC_CACHE = build_nc()
        if not _NC_CACHE.is_finalized():
            _NC_CACHE.finalize()
    return _NC_CACHE


def make_in_maps(inputs):
    import ml_dtypes
    bf16 = ml_dtypes.bfloat16
    f8 = ml_dtypes.float8_e4m3fn

    hs = np.asarray(inputs["hidden_states"], dtype=np.float32)
    w = {k: np.asarray(inputs[k], dtype=np.float32) for k in
         ("w_q", "w_k", "w_v", "w_o", "w_gate", "w_up", "w_down")}
    g_in = np.asarray(inputs["g_in"], dtype=np.float32).reshape(H, 1)
    g_post = np.asarray(inputs["g_post"], dtype=np.float32).reshape(H, 1)
    # large finite negative: exp -> 0 exactly, but stays finite in bf16
    # (fp32 min overflows to -inf when the masked scores pass through bf16)
    neg = np.float32(-30000.0)

    in_maps = []
    for c in range(N_CORES):
        p, hh = c // 2, c % 2
        b, par = p // 2, p % 2
        xb = hs[b]                                    # [S, H]
        xt = np.ascontiguousarray(xb.T).astype(bf16)  # [H, S] bf16
        xo = np.ascontiguousarray(xb[par::2].T)       # [H, TOK]
        xt_own = xo.astype(bf16)
        xt_half = (0.5 * xo).astype(bf16)
        # diagonal causal mask blocks: [128 keys, 4 blocks, QC queries]
        k_idx = np.arange(128)[:, None, None]
        d_idx = np.arange(4)[None, :, None]
        q_idx = np.arange(QC)[None, None, :]
        mskd = np.where(d_idx * 128 + k_idx <= 2 * q_idx + par,
                        np.float32(0.0), neg).astype(np.float32)
        mskd = mskd.reshape(128, 4 * QC)
        cs = slice(hh * HH, (hh + 1) * HH)
        fs = slice(hh * FFH, (hh + 1) * FFH)
        in_maps.append({
            "xtb": xt,
            "xtob": xt_own,
            "xtoh": xt_half,
            "mkd": mskd,
            "wq": np.ascontiguousarray((g_in * w["w_q"])[:, cs] * WS).astype(f8),
            "wk": np.ascontiguousarray((g_in * w["w_k"])[:, cs] * WS).astype(f8),
            "wv": np.ascontiguousarray((g_in * w["w_v"])[:, cs] * WS).astype(f8),
            "wo": np.ascontiguousarray(w["w_o"][cs, :] * WS).astype(f8),
            "wg": np.ascontiguousarray(w["w_gate"][:, fs]).astype(bf16),
            "wu": np.ascontiguousarray(w["w_up"][:, fs]).astype(bf16),
            "wd": np.ascontiguousarray(w["w_down"][fs, :]).astype(bf16),
            "g_in": g_in,
            "g_post": g_post,
            "slot": np.array([[hh, 1 - hh]], dtype=np.uint32),
        })
    return in_maps


def assemble_output(results):
    out = np.empty((B, S, H), dtype=np.float32)
    ht = TOK // 2
    for b in range(B):
        for par in range(2):
            c = (2 * b + par) * 2
            pair_out = np.concatenate(
                [results[c]["out"][:, :ht], results[c + 1]["out"][:, ht:]],
                axis=1).astype(np.float32)
            out[b, par::2, :] = pair_out.T
    return out


def kernel(**inputs):
    nc = _get_nc()
    in_maps = make_in_maps(inputs)
    res = run_bass_kernel_spmd(nc, in_maps, list(range(N_CORES)))
    return assemble_output(res.results)


if __name__ == "__main__":
    import time
    t0 = time.time()
    nc = _get_nc()
    print(f"build+finalize: {time.time()-t0:.1f}s")
